# revision 11
# baseline (speedup 1.0000x reference)
"""BackpropWiSARD forward on 8 Trainium2 NeuronCores.

Strategy (filter-sharded):
  out[b,c] = sum_f mask[c,f] * min_h [data[c, f, idx[b,f,h]] >= 0] + bias[c]

- Host computes the hashed indices idx[b,f,h] (tiny: binarize x, permute,
  XOR-hash), and re-lays-out the 420MB table as [F, E, Cpad=128] bf16 rows so
  one gathered 256B row holds all classes for one (f, e).
- The filter axis F=512 is sharded 64-per-core. Each core gathers, for its
  filters, the B*H rows it needs straight from HBM via GPSIMD dma_gather
  (16 filters = 16384 row-gathers per instruction; row index = f_local*2048+e
  fits int16 exactly), then on-chip: min over the H=4 hash lookups (min and
  binarize commute since x>=0 is monotone), binarize+mask via one
  scalar_tensor_tensor, reduce over filters, accumulate.
- Each core returns a partial [b, c] sum over its filters; the host adds the
  8 partials and the bias (pure unshard/combine work).
"""

import numpy as np
import ml_dtypes

B = 256      # batch
NI = 1024    # num inputs
C = 100      # classes
U = 16       # unit inputs
E = 2048     # unit entries
H = 4        # hashes
BPI = 8      # bits per input
IB = NI * BPI          # 8192
F = IB // U            # 512 filters
NCORES = 8
FPC = F // NCORES      # 64 filters per core
CP = 128               # padded class dim (256B bf16 rows)
GF = 8                 # filters per gather group (HW dma_gather limit: 8192 idxs)
NG = FPC // GF         # 8 groups
NIDX = GF * H * B      # 8192 gathered rows per group
BQ = B // 128          # 2 partition-blocks of the batch

_NC = {}


def _build_nc(reps=1):
    from contextlib import ExitStack
    import concourse.bacc as bacc
    import concourse.mybir as mybir

    nc = bacc.Bacc("TRN2", target_bir_lowering=False, debug=False,
                   num_devices=NCORES, dynamic_dma_scratch_size=32768)
    table = nc.dram_tensor("table", [FPC * E, CP], mybir.dt.bfloat16,
                           kind="ExternalInput")
    idxw = nc.dram_tensor("idxw", [128, NG * (NIDX // 16)], mybir.dt.int16,
                          kind="ExternalInput")
    maskr = nc.dram_tensor("maskr", [128, FPC * CP], mybir.dt.float32,
                           kind="ExternalInput")
    out_acc = nc.dram_tensor("out_acc", [128, BQ * CP], mybir.dt.float32,
                             kind="ExternalOutput")

    mn = mybir.AluOpType.min
    with (
        nc.sbuf_tensor("idx_sb", [128, NG * (NIDX // 16)], mybir.dt.int16) as idx_sb,
        nc.sbuf_tensor("mask_sb", [128, FPC * CP], mybir.dt.float32) as mask_sb,
        nc.sbuf_tensor("gt0", [128, NIDX], mybir.dt.bfloat16) as gt0,
        nc.sbuf_tensor("gt1", [128, NIDX], mybir.dt.bfloat16) as gt1,
        nc.sbuf_tensor("mA", [128, GF * 2 * BQ * CP], mybir.dt.bfloat16) as mA,
        nc.sbuf_tensor("mB", [128, GF * BQ * CP], mybir.dt.bfloat16) as mB,
        nc.sbuf_tensor("respm", [128, GF * BQ * CP], mybir.dt.float32) as respm,
        nc.sbuf_tensor("red", [128, BQ * CP], mybir.dt.float32) as red,
        nc.sbuf_tensor("acc", [128, BQ * CP], mybir.dt.float32) as acc,
        nc.semaphore("s_in") as s_in,
        nc.semaphore("s_v") as s_v,
        nc.semaphore("s_f") as s_f,
        ExitStack() as sem_stack,
    ):
        s_g = [sem_stack.enter_context(nc.semaphore(f"s_g{g}"))
               for g in range(NG)]
        gts = [gt0, gt1]

        # --- gpsimd: input loads + the gather groups ---------------------
        # (Bacc auto-inserts the GPSIMD 'mlp' library load for dma_gather)
        nc.gpsimd.dma_start(idx_sb[:, :], idxw[:, :]).then_inc(s_in, 16)
        nc.gpsimd.dma_start(mask_sb[:, :], maskr[:, :]).then_inc(s_in, 16)
        nc.gpsimd.wait_ge(s_in, 32)
        for rep in range(reps):
            for g in range(NG):
                j = rep * NG + g
                buf = gts[j % 2]
                if j >= 2:
                    # buffer reuse: wait until DVE finished reading it
                    nc.gpsimd.wait_ge(s_v, j - 1)
                nc.gpsimd.dma_gather(
                    buf[:, :].rearrange("p (j c) -> p j c", c=CP),
                    table[g * GF * E:(g + 1) * GF * E, :],
                    idx_sb[:, g * (NIDX // 16):(g + 1) * (NIDX // 16)],
                    NIDX, NIDX, CP, single_packet=False,
                ).then_inc(s_g[g], 16)

        # --- vector: min over h, binarize*mask, reduce over f, accumulate
        for rep in range(reps):
          nc.vector.memset(acc[:, :], 0.0)
          for g in range(NG):
            j = rep * NG + g
            buf = gts[j % 2]
            # gathered tile: [p, f, h, B', c]
            t = buf[:, :].rearrange("p (f h q c) -> p f h q c",
                                    f=GF, h=H, q=BQ)
            a4 = mA[:, :].rearrange("p (f m q c) -> p f m q c",
                                    f=GF, m=2, q=BQ)
            b3 = mB[:, :].rearrange("p (f q c) -> p f q c", f=GF, q=BQ)
            r3 = respm[:, :].rearrange("p (f q c) -> p f q c", f=GF, q=BQ)
            nc.vector.wait_ge(s_g[g], 16 * (rep + 1))
            # min over h in two rounds (4-dim APs only)
            for m in range(2):
                nc.vector.tensor_tensor(
                    a4[:, :, m, :, :], t[:, :, m, :, :], t[:, :, m + 2, :, :], mn)
            nc.vector.drain().then_inc(s_v, 1)  # gt buffer free, mA visible
            nc.vector.tensor_tensor(b3, a4[:, :, 0, :, :], a4[:, :, 1, :, :], mn)
            nc.vector.drain()
            mg = mask_sb[:, g * GF * CP:(g + 1) * GF * CP].rearrange(
                "p (f c) -> p f c", f=GF)
            for q in range(BQ):
                # respm = (min >= 0) * mask
                nc.vector.scalar_tensor_tensor(
                    r3[:, :, q, :], b3[:, :, q, :], 0.0, mg,
                    mybir.AluOpType.is_ge, mybir.AluOpType.mult)
            nc.vector.drain()
            # reduce over f (innermost after permute), then accumulate
            nc.vector.tensor_reduce(
                red[:, :].rearrange("p (q c) -> p q c", q=BQ),
                r3.transpose([0, 2, 3, 1]),
                mybir.AxisListType.X, mybir.AluOpType.add)
            nc.vector.drain()
            nc.vector.tensor_tensor(
                acc[:, :], acc[:, :], red[:, :], mybir.AluOpType.add)
        nc.vector.drain().then_inc(s_f, 1)

        # --- sync: write the partial result back -------------------------
        nc.sync.wait_ge(s_f, 1)
        nc.sync.dma_start(out_acc[:, :], acc[:, :]).then_inc(s_f, 16)
        nc.sync.wait_ge(s_f, 17)
    nc.finalize()
    return nc


def _get_nc(reps=1):
    if reps not in _NC:
        _NC[reps] = _build_nc(reps)
    return _NC[reps]


def _hashed_indices(x, thresholds, hash_values, input_order):
    """idx[b, f, h] in [0, E) — the H3 hash of the binarized inputs."""
    bits = (x[:, :, None] >= thresholds[None, :, :])
    bits = bits.reshape(B, IB)[:, input_order].astype(np.int32)
    hin = bits.reshape(B, F, U)
    prod = hin[:, :, None, :] * hash_values[None, None, :, :].astype(np.int32)
    return np.bitwise_xor.reduce(prod, axis=-1)  # [B, F, H]


def _shard_inputs(idx, data, mask):
    """Per-core input dicts: table slab, wrapped gather indices, repl. mask."""
    data_t = np.zeros((F, E, CP), dtype=ml_dtypes.bfloat16)
    data_t[:, :, :C] = np.transpose(data, (1, 2, 0)).astype(ml_dtypes.bfloat16)
    in_maps = []
    for k in range(NCORES):
        fs = k * FPC
        table_k = np.ascontiguousarray(data_t[fs:fs + FPC]).reshape(FPC * E, CP)
        # gather order within a group: i = f_local*H*B + h*B + b
        r = idx[:, fs:fs + FPC, :].transpose(1, 2, 0)        # [FPC, H, B]
        r = (r.reshape(NG, GF, H, B)
             + (np.arange(GF, dtype=np.int32)[None, :, None, None] * E))
        r = r.reshape(NG, NIDX)
        iw16 = np.zeros((16, NG * (NIDX // 16)), np.int16)
        for g in range(NG):
            iw16[:, g * (NIDX // 16):(g + 1) * (NIDX // 16)] = (
                r[g].reshape(NIDX // 16, 16).T.astype(np.int16))
        iw = np.tile(iw16, (8, 1))  # replicated per Q7 core group
        mk = np.zeros((FPC, CP), np.float32)
        mk[:, :C] = mask[:, fs:fs + FPC].T
        mrep = np.ascontiguousarray(
            np.broadcast_to(mk.reshape(1, FPC * CP), (128, FPC * CP)))
        in_maps.append({"table": table_k, "idxw": iw, "maskr": mrep})
    return in_maps


def kernel(x, thresholds, data, hash_values, input_order, mask, bias):
    import os
    from concourse.bass_utils import run_bass_kernel_spmd

    x = np.asarray(x, np.float32)
    thresholds = np.asarray(thresholds, np.float32)
    data = np.asarray(data, np.float32)
    hash_values = np.asarray(hash_values, np.int32)
    input_order = np.asarray(input_order, np.int32)
    mask = np.asarray(mask, np.float32)
    bias = np.asarray(bias, np.float32)

    idx = _hashed_indices(x, thresholds, hash_values, input_order)
    in_maps = _shard_inputs(idx, data, mask)

    trace = bool(int(os.environ.get("WISARD_TRACE", "0")))
    res = run_bass_kernel_spmd(_get_nc(), in_maps, core_ids=list(range(NCORES)),
                               trace=trace)
    if trace and res.exec_time_ns is not None:
        kernel.last_exec_time_ns = res.exec_time_ns
        kernel.last_trace = res.instructions_and_trace
    kernel.last_results = res

    out = np.zeros((B, CP), np.float32)
    for r in res.results:
        out += r["out_acc"].reshape(128, BQ, CP).transpose(1, 0, 2).reshape(B, CP)
    return out[:, :C] + bias[None, :].astype(np.float32)


# revision 14
# speedup vs baseline: 5.3415x; 5.3415x over previous
"""BackpropWiSARD forward on 8 Trainium2 NeuronCores.

Strategy (filter-sharded):
  out[b,c] = sum_f mask[c,f] * min_h [data[c, f, idx[b,f,h]] >= 0] + bias[c]

- Host computes the hashed indices idx[b,f,h] (tiny: binarize x, permute,
  XOR-hash), and re-lays-out the 420MB table as [F, E, Cpad=128] bf16 rows so
  one gathered 256B row holds all classes for one (f, e).
- The filter axis F=512 is sharded 64-per-core. Each core gathers, for its
  filters, the B*H rows it needs straight from HBM via GPSIMD dma_gather
  (16 filters = 16384 row-gathers per instruction; row index = f_local*2048+e
  fits int16 exactly), then on-chip: min over the H=4 hash lookups (min and
  binarize commute since x>=0 is monotone), binarize+mask via one
  scalar_tensor_tensor, reduce over filters, accumulate.
- Each core returns a partial [b, c] sum over its filters; the host adds the
  8 partials and the bias (pure unshard/combine work).
"""

import numpy as np
import ml_dtypes

B = 256      # batch
NI = 1024    # num inputs
C = 100      # classes
U = 16       # unit inputs
E = 2048     # unit entries
H = 4        # hashes
BPI = 8      # bits per input
IB = NI * BPI          # 8192
F = IB // U            # 512 filters
NCORES = 8
FPC = F // NCORES      # 64 filters per core
CP = 128               # padded class dim (256B bf16 rows)
GF = 8                 # filters per gather group (HW dma_gather limit: 8192 idxs)
NG = FPC // GF         # 8 groups
NIDX = GF * H * B      # 8192 gathered rows per group
BQ = B // 128          # 2 partition-blocks of the batch
NCHUNK = 8             # gather chunks per group (4 SWDGE queues, depth 8)

_NC = {}


def _build_nc(reps=1, variant='full'):
    from contextlib import ExitStack
    import concourse.bacc as bacc
    import concourse.mybir as mybir

    nc = bacc.Bacc("TRN2", target_bir_lowering=False, debug=False,
                   num_devices=NCORES, dynamic_dma_scratch_size=32768,
                   num_swdge_queues=4)
    table = nc.dram_tensor("table", [FPC * E, CP], mybir.dt.bfloat16,
                           kind="ExternalInput")
    idxw = nc.dram_tensor("idxw", [128, NG * (NIDX // 16)], mybir.dt.int16,
                          kind="ExternalInput")
    maskr = nc.dram_tensor("maskr", [128, FPC * CP], mybir.dt.float32,
                           kind="ExternalInput")
    out_acc = nc.dram_tensor("out_acc", [128, BQ * CP], mybir.dt.float32,
                             kind="ExternalOutput")

    mn = mybir.AluOpType.min
    with (
        nc.sbuf_tensor("idx_sb", [128, NG * (NIDX // 16)], mybir.dt.int16) as idx_sb,
        nc.sbuf_tensor("mask_sb", [128, FPC * CP], mybir.dt.float32) as mask_sb,
        nc.sbuf_tensor("gt0", [128, NIDX], mybir.dt.bfloat16) as gt0,
        nc.sbuf_tensor("gt1", [128, NIDX], mybir.dt.bfloat16) as gt1,
        nc.sbuf_tensor("mA", [128, GF * 2 * BQ * CP], mybir.dt.bfloat16) as mA,
        nc.sbuf_tensor("mB", [128, GF * BQ * CP], mybir.dt.bfloat16) as mB,
        nc.sbuf_tensor("respm", [128, GF * BQ * CP], mybir.dt.float32) as respm,
        nc.sbuf_tensor("red", [128, BQ * CP], mybir.dt.float32) as red,
        nc.sbuf_tensor("acc", [128, BQ * CP], mybir.dt.float32) as acc,
        nc.semaphore("s_in") as s_in,
        nc.semaphore("s_v") as s_v,
        nc.semaphore("s_f") as s_f,
        ExitStack() as sem_stack,
    ):
        s_g = [[sem_stack.enter_context(nc.semaphore(f"s_g{g}q{q}"))
                for q in range(4)] for g in range(NG)]
        gts = [gt0, gt1]

        # --- gpsimd: input loads + the gather groups ---------------------
        # (Bacc auto-inserts the GPSIMD 'mlp' library load for dma_gather)
        nc.gpsimd.dma_start(idx_sb[:, :], idxw[:, :]).then_inc(s_in, 16)
        nc.gpsimd.dma_start(mask_sb[:, :], maskr[:, :]).then_inc(s_in, 16)
        nc.gpsimd.wait_ge(s_in, 32)
        gather_reps = reps if variant in ('full', 'gather_only') else 1
        dve_reps = reps if variant in ('full', 'dve_only') else 1
        CH = NIDX // NCHUNK  # idxs per gather chunk
        for rep in range(gather_reps):
            for g in range(NG):
                j = rep * NG + g
                buf = gts[j % 2]
                if variant == 'full' and j >= 2:
                    # buffer reuse: wait until DVE finished reading it
                    nc.gpsimd.wait_ge(s_v, j - 1)
                for ch in range(NCHUNK):
                    # chunk ch covers group idxs [ch*CH, (ch+1)*CH): same
                    # wrapped-idx cols and same gt cols as one big gather
                    nc.gpsimd.dma_gather(
                        buf[:, ch * CH:(ch + 1) * CH].rearrange(
                            "p (j c) -> p j c", c=CP),
                        table[g * GF * E:(g + 1) * GF * E, :],
                        idx_sb[:, g * (NIDX // 16) + ch * (CH // 16):
                               g * (NIDX // 16) + (ch + 1) * (CH // 16)],
                        CH, CH, CP, single_packet=False,
                        queue_num=ch % 4,
                    ).then_inc(s_g[g][ch % 4], 16)

        # --- vector: min over h, binarize*mask, reduce over f, accumulate
        for rep in range(dve_reps):
          nc.vector.memset(acc[:, :], 0.0)
          for g in range(NG):
            j = rep * NG + g
            buf = gts[j % 2] if variant == 'full' else gts[0]
            # gathered tile: [p, f, h, B', c]
            t = buf[:, :].rearrange("p (f h q c) -> p f h q c",
                                    f=GF, h=H, q=BQ)
            a4 = mA[:, :].rearrange("p (f m q c) -> p f m q c",
                                    f=GF, m=2, q=BQ)
            b3 = mB[:, :].rearrange("p (f q c) -> p f q c", f=GF, q=BQ)
            r3 = respm[:, :].rearrange("p (f q c) -> p f q c", f=GF, q=BQ)
            per_q = NCHUNK // 4
            for q in range(4):
                nc.vector.wait_ge(s_g[g][q], 16 * per_q * (rep + 1)
                                  if variant == 'full' else 16 * per_q)
            # min over h in two rounds (4-dim APs only)
            for m in range(2):
                nc.vector.tensor_tensor(
                    a4[:, :, m, :, :], t[:, :, m, :, :], t[:, :, m + 2, :, :], mn)
            nc.vector.drain().then_inc(s_v, 1)  # gt buffer free, mA visible
            nc.vector.tensor_tensor(b3, a4[:, :, 0, :, :], a4[:, :, 1, :, :], mn)
            nc.vector.drain()
            mg = mask_sb[:, g * GF * CP:(g + 1) * GF * CP].rearrange(
                "p (f c) -> p f c", f=GF)
            for q in range(BQ):
                # respm = (min >= 0) * mask
                nc.vector.scalar_tensor_tensor(
                    r3[:, :, q, :], b3[:, :, q, :], 0.0, mg,
                    mybir.AluOpType.is_ge, mybir.AluOpType.mult)
            nc.vector.drain()
            # reduce over f (innermost after permute), then accumulate
            nc.vector.tensor_reduce(
                red[:, :].rearrange("p (q c) -> p q c", q=BQ),
                r3.transpose([0, 2, 3, 1]),
                mybir.AxisListType.X, mybir.AluOpType.add)
            nc.vector.drain()
            nc.vector.tensor_tensor(
                acc[:, :], acc[:, :], red[:, :], mybir.AluOpType.add)
        nc.vector.drain().then_inc(s_f, 1)

        # --- sync: write the partial result back -------------------------
        nc.sync.wait_ge(s_f, 1)
        nc.sync.dma_start(out_acc[:, :], acc[:, :]).then_inc(s_f, 16)
        nc.sync.wait_ge(s_f, 17)
    nc.finalize()
    return nc


def _get_nc(reps=1, variant='full'):
    key = (reps, variant)
    if key not in _NC:
        _NC[key] = _build_nc(reps, variant)
    return _NC[key]


def _hashed_indices(x, thresholds, hash_values, input_order):
    """idx[b, f, h] in [0, E) — the H3 hash of the binarized inputs."""
    bits = (x[:, :, None] >= thresholds[None, :, :])
    bits = bits.reshape(B, IB)[:, input_order].astype(np.int32)
    hin = bits.reshape(B, F, U)
    prod = hin[:, :, None, :] * hash_values[None, None, :, :].astype(np.int32)
    return np.bitwise_xor.reduce(prod, axis=-1)  # [B, F, H]


def _shard_inputs(idx, data, mask):
    """Per-core input dicts: table slab, wrapped gather indices, repl. mask."""
    data_t = np.zeros((F, E, CP), dtype=ml_dtypes.bfloat16)
    data_t[:, :, :C] = np.transpose(data, (1, 2, 0)).astype(ml_dtypes.bfloat16)
    in_maps = []
    for k in range(NCORES):
        fs = k * FPC
        table_k = np.ascontiguousarray(data_t[fs:fs + FPC]).reshape(FPC * E, CP)
        # gather order within a group: i = f_local*H*B + h*B + b
        r = idx[:, fs:fs + FPC, :].transpose(1, 2, 0)        # [FPC, H, B]
        r = (r.reshape(NG, GF, H, B)
             + (np.arange(GF, dtype=np.int32)[None, :, None, None] * E))
        r = r.reshape(NG, NIDX)
        iw16 = np.zeros((16, NG * (NIDX // 16)), np.int16)
        for g in range(NG):
            iw16[:, g * (NIDX // 16):(g + 1) * (NIDX // 16)] = (
                r[g].reshape(NIDX // 16, 16).T.astype(np.int16))
        iw = np.tile(iw16, (8, 1))  # replicated per Q7 core group
        mk = np.zeros((FPC, CP), np.float32)
        mk[:, :C] = mask[:, fs:fs + FPC].T
        mrep = np.ascontiguousarray(
            np.broadcast_to(mk.reshape(1, FPC * CP), (128, FPC * CP)))
        in_maps.append({"table": table_k, "idxw": iw, "maskr": mrep})
    return in_maps


def kernel(x, thresholds, data, hash_values, input_order, mask, bias):
    import os
    from concourse.bass_utils import run_bass_kernel_spmd

    x = np.asarray(x, np.float32)
    thresholds = np.asarray(thresholds, np.float32)
    data = np.asarray(data, np.float32)
    hash_values = np.asarray(hash_values, np.int32)
    input_order = np.asarray(input_order, np.int32)
    mask = np.asarray(mask, np.float32)
    bias = np.asarray(bias, np.float32)

    idx = _hashed_indices(x, thresholds, hash_values, input_order)
    in_maps = _shard_inputs(idx, data, mask)

    trace = bool(int(os.environ.get("WISARD_TRACE", "0")))
    res = run_bass_kernel_spmd(_get_nc(), in_maps, core_ids=list(range(NCORES)),
                               trace=trace)
    if trace and res.exec_time_ns is not None:
        kernel.last_exec_time_ns = res.exec_time_ns
        kernel.last_trace = res.instructions_and_trace
    kernel.last_results = res

    out = np.zeros((B, CP), np.float32)
    for r in res.results:
        out += r["out_acc"].reshape(128, BQ, CP).transpose(1, 0, 2).reshape(B, CP)
    return out[:, :C] + bias[None, :].astype(np.float32)
